# revision 21
# baseline (speedup 1.0000x reference)
"""Benes butterfly network (12 layers, N=4096) on 8 Trainium2 NeuronCores.

Self-contained: takes full inputs, shards batch across 8 cores, runs a
Bass/Tile kernel per core, gathers the full output.

Math: reference layer k is a butterfly with span 2^k:
    h[:, j] <- A_k[j] * h[:, j] + B_k[j] * h[:, j ^ 2^k]
(A_k/B_k extracted from the sparse COO (values, idx_in, idx_out)).

Device decomposition per core (batch shard 512, transposed layout
[col-part, batch-free], 32 col-tiles of 128; x is pre-transposed on the
host so H0 tiles stream in with perfectly coalesced DMA):
  1. phase1: layers 0..8 fused into dense 128x128 block matrices, with
     layer 9's self-scale A9 folded in on the host:
       p1'[t] = sum_{j=0..3} (diag(A9[t]) @ M9[t, t^j]) @ H0[t^j]   (fp32r)
  2. L9 partner via ratio trick: H9[t] = E[t] + (B9[t]/A9[t^4]) * E[t^4]
     where E[t] = evac(p1'[t]) — one ACT/DVE copy + one DVE stt per tile.
  3. L10+L11 (dists 8, 16) fused into the PE out-transpose: for each
     quad {q, q+8, q+16, q+24}: psum[b, 4*128] accumulates 4 matmuls
     stationary=H9[src] b-block, moving=[diag(c[d0<-s])|...] (host-built).
  4. Evacuate pieces + strided DMA back to DRAM rows.
"""
import os
import numpy as np

N = 4096
BATCH = 4096
NLAYERS = 12
NCORES = 8
BSH = BATCH // NCORES      # 512 batch rows per core
T = N // 128               # 32 column tiles

_PROGRAM_CACHE = {}
LAST_EXEC_NS = None


def _extract_ab(values, idx_in, idx_out):
    """Per-layer butterfly coefficients A[k], B[k] (float64 [L, N])."""
    v = np.asarray(values, np.float64)
    ii = np.asarray(idx_in, np.int64)
    io = np.asarray(idx_out, np.int64)
    L, nnz = v.shape
    n = nnz // 2
    A = np.zeros((L, n))
    B = np.zeros((L, n))
    for k in range(L):
        s = 1 << k
        self_m = ii[k] == io[k]
        part_m = ii[k] == (io[k] ^ s)
        if not np.all(self_m | part_m):
            raise ValueError(f"layer {k}: unexpected sparse index structure")
        np.add.at(A[k], io[k][self_m], v[k][self_m])
        np.add.at(B[k], io[k][part_m], v[k][part_m])
    return A, B


def _host_precompute(values, idx_in, idx_out):
    A, B = _extract_ab(values, idx_in, idx_out)
    Ab = A.reshape(NLAYERS, T, 128)
    Bb = B.reshape(NLAYERS, T, 128)
    j = np.arange(128)

    # Block-level composition of layers 0..8: S[t] = {src_tile: 128x128}.
    S = [{t: np.eye(128)} for t in range(T)]
    for k in range(7):  # within-block layers
        s = 1 << k
        for t in range(T):
            W = np.zeros((128, 128))
            W[j, j] = Ab[k, t]
            W[j, j ^ s] = Bb[k, t]
            S[t] = {src: W @ M for src, M in S[t].items()}
    for k in (7, 8):   # cross-block layers, tile distance d
        d = 1 << (k - 7)
        newS = []
        for t in range(T):
            out = {}
            for src, M in S[t].items():
                out[src] = Ab[k, t][:, None] * M
            for src, M in S[t ^ d].items():
                out[src] = out.get(src, 0) + Bb[k, t][:, None] * M
            newS.append(out)
        S = newS

    # fold layer-9 self scale; guard against pathological tiny A9
    A9 = Ab[9].copy()
    tiny = np.abs(A9) < 1e-12
    if tiny.any():
        A9 = np.where(tiny, 1e-12, A9)
    # chunk order follows the device's quad-pair schedule: dest t = qp + 4k
    mst = np.zeros((128, T * 512), np.float32)
    for qp in range(4):
        for k in range(8):
            t = qp + 4 * k
            assert set(S[t].keys()) == {t, t ^ 1, t ^ 2, t ^ 3}
            base = (qp * 8 + k) * 512
            for ji in range(4):
                M = A9[t][:, None] * S[t][t ^ ji]
                mst[:, base + ji * 128: base + (ji + 1) * 128] = (
                    M.T.astype(np.float32)
                )

    # L9 partner ratio scales rB9[t] = B9[t] / A9[t^4]
    scales = np.zeros((128, 32), np.float32)
    for t in range(T):
        scales[:, t] = (Bb[9, t] / A9[t ^ 4]).astype(np.float32)

    # out-transpose movings, quad-major: for quad q, slot si (src s=q+8*si),
    # block k holds diag(c[q+8k <- s]) where c are the fused L10*L11
    # coefficients acting on H9 (post-L9 state)
    movd = np.zeros((128, T * 512), np.float32)
    for s in range(T):
        q = s & 7
        si = s >> 3
        for k in range(4):
            d = q + 8 * k
            if s == d:
                c = Ab[11, d] * Ab[10, d]
            elif s == (d ^ 8):
                c = Ab[11, d] * Bb[10, d]
            elif s == (d ^ 16):
                c = Bb[11, d] * Ab[10, d ^ 16]
            else:  # s == d ^ 24
                c = Bb[11, d] * Bb[10, d ^ 16]
            movd[j, q * 2048 + si * 512 + k * 128 + j] = c.astype(np.float32)
    return mst, scales, movd


def _build_program():
    import concourse.bass as bass
    import concourse.mybir as mybir
    import concourse.tile as tile
    from concourse import bacc

    f32 = mybir.dt.float32
    f32r = mybir.dt.float32r
    mult = mybir.AluOpType.mult
    add = mybir.AluOpType.add

    nc = bacc.Bacc("TRN2", target_bir_lowering=False, debug=False)
    # x pre-transposed on host: [N, BSH] (column-major over batch shard)
    xT_ap = nc.dram_tensor("xT", [N, BSH], f32r, kind="ExternalInput").ap()
    mst_ap = nc.dram_tensor("mst", [128, T * 512], f32r, kind="ExternalInput").ap()
    sc_ap = nc.dram_tensor("scales", [128, 32], f32, kind="ExternalInput").ap()
    mov_ap = nc.dram_tensor("movd", [128, T * 512], f32r, kind="ExternalInput").ap()
    out_ap = nc.dram_tensor("out", [BSH, N], f32, kind="ExternalOutput").ap()

    with tile.TileContext(nc) as tc:
        with (
            tc.tile_pool(name="const", bufs=1) as constp,
            tc.tile_pool(name="h0", bufs=8) as h0p,
            tc.tile_pool(name="mstp", bufs=6) as mstp,
            tc.tile_pool(name="H", bufs=22) as Hp,
            tc.tile_pool(name="mov", bufs=4) as movp,
            tc.tile_pool(name="piece", bufs=6) as piecep,
            tc.tile_pool(name="ps", bufs=8, space="PSUM") as psp,
        ):
            # first mst chunk leads the ACT ring so phase1 starts early
            msts = {}
            msts[0] = mstp.tile([128, 512], f32r, tag="mst", name="mst_c0")
            nc.scalar.dma_start(msts[0][:], mst_ap[:, 0:512])
            sc = constp.tile([128, 32], f32)
            nc.scalar.dma_start(sc[:], sc_ap[:])

            # H0 tiles via 1MB 3D-strided DMAs, resident for all 4 passes:
            # H0cat[kb][p, lt*512+b] = xT[(4*kb+lt)*128 + p, b]
            H0cat = {}
            for kb in range(8):
                h0c = h0p.tile([128, 2048], f32r, tag="h0", name=f"h0c_{kb}")
                src = xT_ap[kb * 512:(kb + 1) * 512, :].rearrange(
                    "(lt p) b -> p lt b", lt=4, p=128
                )
                nc.sync.dma_start(h0c[:].rearrange("p (lt b) -> p lt b", lt=4), src)
                H0cat[kb] = h0c

            # quad-pair passes: dests {qp + 4k}, then L9, then quads qp/qp+4,
            # so the output stream starts after ~1/4 of phase1.
            for qp in range(4):
                E, H9 = {}, {}
                for k in range(8):
                    t = qp + 4 * k
                    ci = qp * 8 + k
                    if ci not in msts:
                        msts[ci] = mstp.tile(
                            [128, 512], f32r, tag="mst", name=f"mst_c{ci}"
                        )
                        nc.scalar.dma_start(
                            msts[ci][:], mst_ap[:, ci * 512:(ci + 1) * 512]
                        )
                    mchunk = msts[ci]
                    p1 = psp.tile([128, 512], f32, name=f"p1_{t}", tag="ps")
                    for ji in range(4):
                        nc.tensor.matmul(
                            p1[:],
                            mchunk[:, ji * 128:(ji + 1) * 128],
                            H0cat[k][:, (qp ^ ji) * 512:((qp ^ ji) + 1) * 512],
                            start=(ji == 0), stop=(ji == 3),
                        )
                    E[t] = Hp.tile([128, 512], f32r, tag="H", name=f"E_{t}")
                    if t % 2 == 0:
                        nc.scalar.copy(E[t][:], p1[:])
                    else:
                        nc.vector.tensor_copy(E[t][:], p1[:])
                # L9: H9[t] = E[t] + rB9[t] * E[t^4]  (t^4 stays in this pass)
                for k in range(8):
                    t = qp + 4 * k
                    H9[t] = Hp.tile([128, 512], f32r, tag="H", name=f"H9_{t}")
                    nc.vector.scalar_tensor_tensor(
                        H9[t][:], E[t ^ 4][:], sc[:, t:t + 1], E[t][:],
                        op0=mult, op1=add,
                    )
                # out-transpose + L10 + L11 for quads qp and qp+4
                for q in (qp, qp + 4):
                    srcs = [q, q + 8, q + 16, q + 24]
                    mv = movp.tile([128, 2048], f32r, tag="mov", name=f"mov_{q}")
                    nc.scalar.dma_start(mv[:], mov_ap[:, q * 2048:(q + 1) * 2048])
                    for bb in range(4):
                        pq = psp.tile([128, 512], f32, tag="ps", name=f"pq_{q}_{bb}")
                        for si, s in enumerate(srcs):
                            nc.tensor.matmul(
                                pq[:], H9[s][:, bb * 128:(bb + 1) * 128],
                                mv[:, si * 512:(si + 1) * 512],
                                start=(si == 0), stop=(si == 3),
                            )
                        piece = piecep.tile([128, 512], f32, tag="piece")
                        if (q + bb) % 2 == 0:
                            nc.scalar.copy(piece[:], pq[:])
                        else:
                            nc.vector.tensor_copy(piece[:], pq[:])
                        dst = out_ap[bb * 128:(bb + 1) * 128, :].rearrange(
                            "p (k t c) -> p k t c", k=4, t=8, c=128
                        )[:, :, q, :]
                        src = piece[:].rearrange("p (k c) -> p k c", k=4, c=128)
                        nc.sync.dma_start(dst, src)

    nc.compile()
    return nc


def kernel(x, values, idx_in, idx_out):
    global LAST_EXEC_NS
    from concourse.bass_utils import run_bass_kernel_spmd

    x = np.asarray(x, np.float32)
    assert x.shape == (BATCH, N), x.shape
    mst, scales, movd = _host_precompute(values, idx_in, idx_out)
    xT = np.ascontiguousarray(x.T)

    if "prog" not in _PROGRAM_CACHE:
        _PROGRAM_CACHE["prog"] = _build_program()
    nc = _PROGRAM_CACHE["prog"]

    in_maps = [
        {
            "xT": np.ascontiguousarray(xT[:, i * BSH:(i + 1) * BSH]),
            "mst": mst,
            "scales": scales,
            "movd": movd,
        }
        for i in range(NCORES)
    ]
    res = run_bass_kernel_spmd(nc, in_maps, core_ids=list(range(NCORES)))
    if os.environ.get("BENES_TRACE"):
        tres = run_bass_kernel_spmd(
            nc, in_maps, core_ids=list(range(NCORES)), trace=True
        )
        LAST_EXEC_NS = tres.exec_time_ns
        _PROGRAM_CACHE["profile_json"] = tres.profile_json
    out = np.empty((BATCH, N), np.float32)
    for i in range(NCORES):
        out[i * BSH:(i + 1) * BSH] = res.results[i]["out"]
    return out


# revision 24
# speedup vs baseline: 1.3212x; 1.3212x over previous
"""Benes butterfly network (12 layers, N=4096) on 8 Trainium2 NeuronCores.

Self-contained: takes full inputs, shards batch across 8 cores, runs a
Bass/Tile kernel per core, gathers the full output.

Math: reference layer k is a butterfly with span 2^k:
    h[:, j] <- A_k[j] * h[:, j] + B_k[j] * h[:, j ^ 2^k]
(A_k/B_k extracted from the sparse COO (values, idx_in, idx_out)).

Device decomposition per core (batch shard 512, transposed layout
[col-part, batch-free], 32 col-tiles of 128; x is pre-transposed on the
host so H0 tiles stream in with perfectly coalesced DMA):
  1. phase1: layers 0..8 fused into dense 128x128 block matrices, with
     layer 9's self-scale A9 folded in on the host:
       p1'[t] = sum_{j=0..3} (diag(A9[t]) @ M9[t, t^j]) @ H0[t^j]   (fp32r)
  2. L9 partner via ratio trick: H9[t] = E[t] + (B9[t]/A9[t^4]) * E[t^4]
     where E[t] = evac(p1'[t]) — one ACT/DVE copy + one DVE stt per tile.
  3. L10+L11 (dists 8, 16) fused into the PE out-transpose: for each
     quad {q, q+8, q+16, q+24}: psum[b, 4*128] accumulates 4 matmuls
     stationary=H9[src] b-block, moving=[diag(c[d0<-s])|...] (host-built).
  4. Evacuate pieces + strided DMA back to DRAM rows.
"""
import os
import numpy as np

N = 4096
BATCH = 4096
NLAYERS = 12
NCORES = 8
BSH = BATCH // NCORES      # 512 batch rows per core
T = N // 128               # 32 column tiles

_PROGRAM_CACHE = {}
LAST_EXEC_NS = None


def _extract_ab(values, idx_in, idx_out):
    """Per-layer butterfly coefficients A[k], B[k] (float64 [L, N])."""
    v = np.asarray(values, np.float64)
    ii = np.asarray(idx_in, np.int64)
    io = np.asarray(idx_out, np.int64)
    L, nnz = v.shape
    n = nnz // 2
    A = np.zeros((L, n))
    B = np.zeros((L, n))
    for k in range(L):
        s = 1 << k
        self_m = ii[k] == io[k]
        part_m = ii[k] == (io[k] ^ s)
        if not np.all(self_m | part_m):
            raise ValueError(f"layer {k}: unexpected sparse index structure")
        np.add.at(A[k], io[k][self_m], v[k][self_m])
        np.add.at(B[k], io[k][part_m], v[k][part_m])
    return A, B


def _host_precompute(values, idx_in, idx_out):
    A, B = _extract_ab(values, idx_in, idx_out)
    Ab = A.reshape(NLAYERS, T, 128)
    Bb = B.reshape(NLAYERS, T, 128)
    j = np.arange(128)

    # Block-level composition of layers 0..8: S[t] = {src_tile: 128x128}.
    S = [{t: np.eye(128)} for t in range(T)]
    for k in range(7):  # within-block layers
        s = 1 << k
        for t in range(T):
            W = np.zeros((128, 128))
            W[j, j] = Ab[k, t]
            W[j, j ^ s] = Bb[k, t]
            S[t] = {src: W @ M for src, M in S[t].items()}
    for k in (7, 8):   # cross-block layers, tile distance d
        d = 1 << (k - 7)
        newS = []
        for t in range(T):
            out = {}
            for src, M in S[t].items():
                out[src] = Ab[k, t][:, None] * M
            for src, M in S[t ^ d].items():
                out[src] = out.get(src, 0) + Bb[k, t][:, None] * M
            newS.append(out)
        S = newS

    # fold layer-9 self scale; guard against pathological tiny A9
    A9 = Ab[9].copy()
    tiny = np.abs(A9) < 1e-12
    if tiny.any():
        A9 = np.where(tiny, 1e-12, A9)
    mst = np.zeros((128, T * 512), np.float32)
    for t in range(T):
        assert set(S[t].keys()) == {t, t ^ 1, t ^ 2, t ^ 3}
        for ji in range(4):
            M = A9[t][:, None] * S[t][t ^ ji]
            mst[:, t * 512 + ji * 128: t * 512 + (ji + 1) * 128] = (
                M.T.astype(np.float32)
            )

    # L9 partner ratio scales rB9[t] = B9[t] / A9[t^4]
    scales = np.zeros((128, 32), np.float32)
    for t in range(T):
        scales[:, t] = (Bb[9, t] / A9[t ^ 4]).astype(np.float32)

    # out-transpose movings, quad-major: for quad q, slot si (src s=q+8*si),
    # block k holds diag(c[q+8k <- s]) where c are the fused L10*L11
    # coefficients acting on H9 (post-L9 state)
    movd = np.zeros((128, T * 512), np.float32)
    for s in range(T):
        q = s & 7
        si = s >> 3
        for k in range(4):
            d = q + 8 * k
            if s == d:
                c = Ab[11, d] * Ab[10, d]
            elif s == (d ^ 8):
                c = Ab[11, d] * Bb[10, d]
            elif s == (d ^ 16):
                c = Bb[11, d] * Ab[10, d ^ 16]
            else:  # s == d ^ 24
                c = Bb[11, d] * Bb[10, d ^ 16]
            movd[j, q * 2048 + si * 512 + k * 128 + j] = c.astype(np.float32)
    return mst, scales, movd


def _build_program():
    import concourse.bass as bass
    import concourse.mybir as mybir
    import concourse.tile as tile
    from concourse import bacc

    f32 = mybir.dt.float32
    f32r = mybir.dt.float32r
    mult = mybir.AluOpType.mult
    add = mybir.AluOpType.add

    nc = bacc.Bacc("TRN2", target_bir_lowering=False, debug=False)
    # x pre-transposed on host: [N, BSH] (column-major over batch shard)
    xT_ap = nc.dram_tensor("xT", [N, BSH], f32r, kind="ExternalInput").ap()
    mst_ap = nc.dram_tensor("mst", [128, T * 512], f32r, kind="ExternalInput").ap()
    sc_ap = nc.dram_tensor("scales", [128, 32], f32, kind="ExternalInput").ap()
    mov_ap = nc.dram_tensor("movd", [128, T * 512], f32r, kind="ExternalInput").ap()
    out_ap = nc.dram_tensor("out", [BSH, N], f32, kind="ExternalOutput").ap()

    with tile.TileContext(nc) as tc:
        with (
            tc.tile_pool(name="const", bufs=1) as constp,
            tc.tile_pool(name="h0", bufs=8) as h0p,
            tc.tile_pool(name="mstp", bufs=6) as mstp,
            tc.tile_pool(name="H", bufs=40) as Hp,
            tc.tile_pool(name="mov", bufs=4) as movp,
            tc.tile_pool(name="piece", bufs=6) as piecep,
            tc.tile_pool(name="ps", bufs=8, space="PSUM") as psp,
        ):
            # first mst chunk leads the ACT ring so phase1 starts early
            msts = {}
            msts[0] = mstp.tile([128, 512], f32r, tag="mst", name="mst_c0")
            nc.scalar.dma_start(msts[0][:], mst_ap[:, 0:512])
            sc = constp.tile([128, 32], f32)
            nc.scalar.dma_start(sc[:], sc_ap[:])

            # H0 tiles via 1MB 3D-strided DMAs, resident for all 4 passes:
            # H0cat[kb][p, lt*512+b] = xT[(4*kb+lt)*128 + p, b]
            H0cat = {}
            for kb in range(8):
                h0c = h0p.tile([128, 2048], f32r, tag="h0", name=f"h0c_{kb}")
                src = xT_ap[kb * 512:(kb + 1) * 512, :].rearrange(
                    "(lt p) b -> p lt b", lt=4, p=128
                )
                nc.sync.dma_start(h0c[:].rearrange("p (lt b) -> p lt b", lt=4), src)
                H0cat[kb] = h0c

            E, H9 = {}, {}
            for qt in range(8):
                for lt in range(4):
                    t = 4 * qt + lt
                    if t not in msts:
                        msts[t] = mstp.tile(
                            [128, 512], f32r, tag="mst", name=f"mst_c{t}"
                        )
                        nc.scalar.dma_start(
                            msts[t][:], mst_ap[:, t * 512:(t + 1) * 512]
                        )
                    mchunk = msts[t]
                    p1 = psp.tile([128, 512], f32, name=f"p1_{t}", tag="ps")
                    for ji in range(4):
                        nc.tensor.matmul(
                            p1[:],
                            mchunk[:, ji * 128:(ji + 1) * 128],
                            H0cat[qt][:, (lt ^ ji) * 512:((lt ^ ji) + 1) * 512],
                            start=(ji == 0), stop=(ji == 3),
                        )
                    E[t] = Hp.tile([128, 512], f32r, tag="H", name=f"E_{t}")
                    if t % 2 == 0:
                        nc.scalar.copy(E[t][:], p1[:])
                    else:
                        nc.vector.tensor_copy(E[t][:], p1[:])
                if qt % 2 == 1:
                    # L9 for the finished 8-group: H9[t] = E[t] + rB9[t]*E[t^4]
                    g = qt // 2
                    for t in range(8 * g, 8 * g + 8):
                        H9[t] = Hp.tile([128, 512], f32r, tag="H", name=f"H9_{t}")
                        nc.vector.scalar_tensor_tensor(
                            H9[t][:], E[t ^ 4][:], sc[:, t:t + 1], E[t][:],
                            op0=mult, op1=add,
                        )

            # ---- out-transpose + L10 + L11 (quads) ----
            for q in range(8):
                srcs = [q, q + 8, q + 16, q + 24]
                mv = movp.tile([128, 2048], f32r, tag="mov", name=f"mov_{q}")
                nc.scalar.dma_start(mv[:], mov_ap[:, q * 2048:(q + 1) * 2048])
                for bb in range(4):
                    pq = psp.tile([128, 512], f32, tag="ps", name=f"pq_{q}_{bb}")
                    for si, s in enumerate(srcs):
                        nc.tensor.matmul(
                            pq[:], H9[s][:, bb * 128:(bb + 1) * 128],
                            mv[:, si * 512:(si + 1) * 512],
                            start=(si == 0), stop=(si == 3),
                        )
                    piece = piecep.tile([128, 512], f32, tag="piece")
                    if (q + bb) % 2 == 0:
                        nc.scalar.copy(piece[:], pq[:])
                    else:
                        nc.vector.tensor_copy(piece[:], pq[:])
                    dst = out_ap[bb * 128:(bb + 1) * 128, :].rearrange(
                        "p (k t c) -> p k t c", k=4, t=8, c=128
                    )[:, :, q, :]
                    src = piece[:].rearrange("p (k c) -> p k c", k=4, c=128)
                    nc.sync.dma_start(dst, src)

    nc.compile()
    return nc


def kernel(x, values, idx_in, idx_out):
    global LAST_EXEC_NS
    from concourse.bass_utils import run_bass_kernel_spmd

    x = np.asarray(x, np.float32)
    assert x.shape == (BATCH, N), x.shape
    mst, scales, movd = _host_precompute(values, idx_in, idx_out)
    xT = np.ascontiguousarray(x.T)

    if "prog" not in _PROGRAM_CACHE:
        _PROGRAM_CACHE["prog"] = _build_program()
    nc = _PROGRAM_CACHE["prog"]

    in_maps = [
        {
            "xT": np.ascontiguousarray(xT[:, i * BSH:(i + 1) * BSH]),
            "mst": mst,
            "scales": scales,
            "movd": movd,
        }
        for i in range(NCORES)
    ]
    res = run_bass_kernel_spmd(nc, in_maps, core_ids=list(range(NCORES)))
    if os.environ.get("BENES_TRACE"):
        tres = run_bass_kernel_spmd(
            nc, in_maps, core_ids=list(range(NCORES)), trace=True
        )
        LAST_EXEC_NS = tres.exec_time_ns
        _PROGRAM_CACHE["profile_json"] = tres.profile_json
    out = np.empty((BATCH, N), np.float32)
    for i in range(NCORES):
        out[i * BSH:(i + 1) * BSH] = res.results[i]["out"]
    return out
